# revision 22
# baseline (speedup 1.0000x reference)
"""GCNConv Bass kernel for Trainium2, 8-core SPMD.

Math (reference): out = D^-1/2 (A + I) D^-1/2 (x @ W) + b.
Aggregation commutes with the linear layer; with xs = dinv * x pre-scaled:
    out[d] = dinv[d] * ( sum_{e: dst(e)=d} xs[src(e)] + xs[d] ) @ W + b

Sharding: 256-dst windows are bin-packed across the 8 cores (local-search
on per-bank group cost).  SWDGE descriptor emission on GpSimd (~2.9ns per
gathered row, 4 queues) is the wall, so group count is minimized with a
two-tier scheme per (band, psum-bank of 512 dsts): per-128-subslot PURE
groups (one-hot width 128) plus bank-wide MIXED groups (width 512) that
absorb every subslot's leftovers - total groups hit the per-bank lower
bound ceil(max_core bank_edges/128).  Group counts are shared across
cores (SPMD single program); padding uses idx 0 + dst_rel=-1 (the
negative-index skip path is ~2.6x slower, so pads gather row 0 and the
one-hot drops them).

Slots are processed in sextets (6 slots = 12 subslots = 3 PSUM banks)
whose [64f, 1536d] PSUM accumulator stays resident across all 4 source
bands (bands exist because dma_gather indices are int16; 25000 rows of
256B each).  Each 2KB PSUM bank is one accumulation group (bank zeroing
granule): the bank's first group in program order carries start=True.

xs is stored bf16 in a [N, 128] row (left half = features, right half
zero) so each gather descriptor is 256B (hardware minimum) yet messages
arrive in bf16 directly - scatter matmuls run at bf16 PE rates.  One-hot
compares run on f32 iota/dst_rel (exact integers beyond 256) with bf16
output.  Self loops live in agg_sb's DMA-loaded initial value (xsloc^T);
bank flushes add the PSUM cell on top.

Final per 128-window: agg^T @ W matmul (fp32), fused dinv scale + bias
via scalar_tensor_tensor, output DMA.
"""

import numpy as np
import ml_dtypes

BF16 = ml_dtypes.bfloat16

N_CORES = 8
AW = 256  # window width for core assignment / output
SW = 128  # subslot width = pure one-hot width = final window
MW = 512  # mixed one-hot width = PSUM bank width
P = 128
SEXT = 6  # 256-slots per PSUM residency (= 12 subslots = 3 banks)
BANKSUB = 4  # subslots per 2KB PSUM bank (zeroing granule)
NBANK_SEXT = 3
BAND_ROWS = 25000  # int16 gather index limit (256B rows)
KG = 32  # max groups (of 128 edges) per dma_gather call
OH_B = 16  # pure groups per batched one-hot DVE instruction
OH_BM = 4  # mixed groups per batch
N_QUEUES = 4


def _assign_windows(cnt_w, n_cores, bandcnt=None, iters=60000):
    """Bin-pack 256-windows across cores by edge count; refine with a
    local search minimizing sum over (bank = adjacent slot pair, band) of
    ceil(max_core bank_count / 128) (the bank-mixed group lower bound)."""
    nw = len(cnt_w)
    nslots = -(-nw // n_cores)
    order = np.argsort(-cnt_w, kind="stable")
    win_of = np.full((n_cores, nslots), -1, np.int64)
    for j in range(nslots):
        chunk = order[j * n_cores:(j + 1) * n_cores]
        for i, w in enumerate(chunk):
            win_of[(i + j) % n_cores, j] = w

    if bandcnt is None:
        return win_of

    nbands = bandcnt.shape[1]
    bc = np.vstack([bandcnt, np.zeros((1, nbands), bandcnt.dtype)])
    assign = np.where(win_of >= 0, win_of, nw)  # [n_cores, nslots]
    npair = -(-nslots // 2)

    def pair_cost(jp, a=None):
        a = assign if a is None else a
        j0, j1 = 2 * jp, 2 * jp + 1
        tot = bc[a[:, j0]]
        if j1 < nslots:
            tot = tot + bc[a[:, j1]]
        return int(np.ceil(tot.max(axis=0) / P).sum())

    costs = np.array([pair_cost(jp) for jp in range(npair)])
    rng = np.random.default_rng(12345)
    ja = rng.integers(0, nslots, iters)
    jb = rng.integers(0, nslots, iters)
    ca = rng.integers(0, n_cores, iters)
    cb = rng.integers(0, n_cores, iters)
    for t in range(iters):
        j1, j2, c1, c2 = ja[t], jb[t], ca[t], cb[t]
        p1, p2 = j1 // 2, j2 // 2
        if p1 == p2:
            continue
        a = assign.copy()
        a[c1, j1], a[c2, j2] = assign[c2, j2], assign[c1, j1]
        n1, n2 = pair_cost(p1, a), pair_cost(p2, a)
        if n1 + n2 < costs[p1] + costs[p2]:
            assign[c1, j1], assign[c2, j2] = a[c1, j1], a[c2, j2]
            costs[p1], costs[p2] = n1, n2
    return np.where(assign == nw, -1, assign)


def _prepare(x, edge_index, W, b, n_cores, band_rows):
    N, C = x.shape
    n_bands = -(-N // band_rows)
    nw = -(-N // AW)  # global 256-dst windows
    nslots = -(-nw // n_cores)
    nsub = nslots * 2
    nbank = -(-nsub // BANKSUB)
    nsext = -(-nslots // SEXT)
    npc_out = nslots * AW

    row = np.asarray(edge_index[0], dtype=np.int64)
    col = np.asarray(edge_index[1], dtype=np.int64)

    deg = np.bincount(col, minlength=N) + 1  # +1 self loop
    dinv = (1.0 / np.sqrt(deg)).astype(np.float32)
    xs = np.asarray(x, dtype=np.float32) * dinv[:, None]

    w_glob = col // AW
    cnt_w = np.bincount(w_glob, minlength=nw)
    bandcnt = np.bincount(w_glob * n_bands + row // band_rows,
                          minlength=nw * n_bands).reshape(nw, n_bands)
    win_of = _assign_windows(cnt_w, n_cores, bandcnt)
    core_of_w = np.zeros(nw, np.int64)
    slot_of_w = np.zeros(nw, np.int64)
    for c in range(n_cores):
        for j in range(nslots):
            w = win_of[c, j]
            if w >= 0:
                core_of_w[w] = c
                slot_of_w[w] = j

    core = core_of_w[w_glob]
    rel256 = col - w_glob * AW
    sub = slot_of_w[w_glob] * 2 + rel256 // SW
    rel128 = (rel256 % SW).astype(np.float32)
    band = row // band_rows
    sext = sub // (2 * SEXT)

    order = np.lexsort((row, sub, band, sext, core))
    band_s = band[order]
    rel_row_s = (row[order] - band_s * band_rows).astype(np.int16)
    dr_s = rel128[order]
    sub_s = sub[order]

    # edge ranges per (core, band, sub) cell, in sorted order:
    # rank = (core, sext, band, sub) program order
    cell_rank = np.zeros((n_bands, nsub), np.int64)
    r = 0
    for s in range(nsext):
        for bb in range(n_bands):
            for j in range(s * 2 * SEXT, min((s + 1) * 2 * SEXT, nsub)):
                cell_rank[bb, j] = r
                r += 1
    n_cells = n_bands * nsub
    key = core[order] * n_cells + cell_rank[band_s, sub_s]
    cnt_rank = np.bincount(key, minlength=n_cores * n_cells).reshape(
        n_cores, n_cells)
    estart = np.zeros(n_cores * n_cells + 1, np.int64)
    estart[1:] = np.cumsum(cnt_rank.reshape(-1))

    def cell_cnt(c, bb, j):
        return cnt_rank[c, cell_rank[bb, j]]

    # per (band, bank): choose pure group counts per subslot + mixed count
    # minimizing total groups (tie: max pure share)
    from itertools import product
    G_pure = np.zeros((n_bands, nsub), np.int64)
    G_mix = np.zeros((n_bands, nbank), np.int64)
    for bb in range(n_bands):
        for bk in range(nbank):
            subs = list(range(bk * BANKSUB, min((bk + 1) * BANKSUB, nsub)))
            c_mat = np.array([[cell_cnt(c, bb, j) for j in subs]
                              for c in range(n_cores)])
            best = None
            cands = [range(int(c_mat[:, i].min()) // P,
                           int(c_mat[:, i].max()) // P + 1)
                     for i in range(len(subs))]
            for gp in product(*cands):
                spill = np.maximum(c_mat - np.array(gp) * P, 0).sum(axis=1)
                gm = int(np.ceil(spill.max() / P))
                tot = sum(gp) + gm
                sc = (tot, -sum(gp))
                if best is None or sc < best[0]:
                    best = (sc, gp, gm)
            G_pure[bb, subs] = best[1]
            G_mix[bb, bk] = best[2]

    # global group order: (sext, band, bank, [pure sub0..sub3, mixed])
    # garr: (band, kind, psum_off, width, start, stop) per group
    garr = []
    gp_start = np.zeros((n_bands, nsub), np.int64)
    gm_start = np.zeros((n_bands, nbank), np.int64)
    gsext = []  # first group index of each sextet
    pair_first = {}
    pair_last = {}
    for s in range(nsext):
        gsext.append(len(garr))
        for bb in range(n_bands):
            # pure groups of every bank first (one long equal-width run
            # for DVE batching), then the mixed groups
            for bk in range(s * NBANK_SEXT,
                            min((s + 1) * NBANK_SEXT, nbank)):
                bloc = bk - s * NBANK_SEXT
                pr = (s, bloc)
                for j in range(bk * BANKSUB,
                               min((bk + 1) * BANKSUB, nsub)):
                    gp_start[bb, j] = len(garr)
                    off = (j - s * 2 * SEXT) * SW
                    for k in range(int(G_pure[bb, j])):
                        pair_first.setdefault(pr, len(garr))
                        pair_last[pr] = len(garr)
                        garr.append([bb, 0, off, SW, False, False])
            for bk in range(s * NBANK_SEXT,
                            min((s + 1) * NBANK_SEXT, nbank)):
                bloc = bk - s * NBANK_SEXT
                pr = (s, bloc)
                gm_start[bb, bk] = len(garr)
                for k in range(int(G_mix[bb, bk])):
                    pair_first.setdefault(pr, len(garr))
                    pair_last[pr] = len(garr)
                    garr.append([bb, 1, bloc * MW, MW, False, False])
    gsext.append(len(garr))
    gtot = len(garr)
    for pr, gi in pair_first.items():
        garr[gi][4] = True
    for pr, gi in pair_last.items():
        garr[gi][5] = True
    started_banks = set(pair_first)

    # calls: chunks of <=KG groups within one (sext, band)
    calls = []
    for s in range(nsext):
        for bb in range(n_bands):
            bk0 = s * NBANK_SEXT
            bk1 = min((s + 1) * NBANK_SEXT, nbank)
            j0 = bk0 * BANKSUB
            g0 = int(gp_start[bb, j0]) if G_pure[bb, j0] > 0 or True else 0
            # first group of (s, bb) = gp_start of first sub (even if its
            # pure count is 0, gp_start still marks the position)
            gend = (int(gm_start[bb, bk1 - 1]) + int(G_mix[bb, bk1 - 1]))
            g = g0
            while g < gend:
                ng = min(KG, gend - g)
                calls.append((bb, g, ng))
                g += ng

    xs_pack = np.zeros((N, 2 * C), dtype=BF16)
    xs_pack[:, :C] = xs.astype(BF16)
    W32 = np.ascontiguousarray(np.asarray(W, dtype=np.float32))
    b32 = np.broadcast_to(np.asarray(b, dtype=np.float32), (P, C)).copy()

    in_maps = []
    for c in range(n_cores):
        # pad idx 0: valid row, gathered but discarded (dst_rel=-1); the
        # negative-index skip path is slower than a real descriptor
        ridx = np.zeros((gtot, P), np.int16)
        drel = np.full((gtot, P), -1.0, np.float32)
        for bb in range(n_bands):
            for bk in range(nbank):
                spill_r = []
                spill_d = []
                for j in range(bk * BANKSUB,
                               min((bk + 1) * BANKSUB, nsub)):
                    k = c * n_cells + cell_rank[bb, j]
                    e0, e1 = estart[k], estart[k + 1]
                    cap = int(G_pure[bb, j]) * P
                    npure = min(int(e1 - e0), cap)
                    g0 = gp_start[bb, j]
                    ridx[g0:g0 + cap // P].reshape(-1)[:npure] = \
                        rel_row_s[e0:e0 + npure]
                    drel[g0:g0 + cap // P].reshape(-1)[:npure] = \
                        dr_s[e0:e0 + npure]
                    if e0 + npure < e1:
                        spill_r.append(rel_row_s[e0 + npure:e1])
                        spill_d.append(dr_s[e0 + npure:e1]
                                       + (j % BANKSUB) * SW)
                gm0, gmn = gm_start[bb, bk], int(G_mix[bb, bk])
                if spill_r:
                    sr = np.concatenate(spill_r)
                    sd = np.concatenate(spill_d)
                    assert len(sr) <= gmn * P, (len(sr), gmn)
                    ridx[gm0:gm0 + gmn].reshape(-1)[:len(sr)] = sr
                    drel[gm0:gm0 + gmn].reshape(-1)[:len(sd)] = sd
        gidx = np.tile(
            ridx.reshape(gtot, 8, 16).transpose(2, 0, 1).reshape(16, gtot * 8),
            (8, 1)).astype(np.int16)

        # agg init = self-loop contribution xs^T for this core's windows
        xslocT = np.zeros((C, npc_out), np.float32)
        dloc = np.zeros(nsub * P, np.float32)
        for j in range(nslots):
            w = win_of[c, j]
            if w < 0:
                continue
            lo = w * AW
            ws = min(AW, N - lo)
            xslocT[:, j * AW:j * AW + ws] = xs[lo:lo + ws].T
            dloc[j * AW:j * AW + ws] = dinv[lo:lo + ws]
        dinvloc = np.ascontiguousarray(dloc.reshape(nsub, P).T)

        in_maps.append({
            "xs": xs_pack,
            "gidx": np.ascontiguousarray(gidx),
            "dstrel": np.ascontiguousarray(drel.T),
            "xslocT": xslocT,
            "dinvloc": dinvloc,
            "wmat": W32,
            "bias": b32,
        })
    meta = {
        "garr": [tuple(g) for g in garr],
        "calls": calls,
        "gsext": gsext,
        "started_banks": started_banks,
        "gtot": gtot,
        "nslots": nslots,
        "nsub": nsub,
        "nbank": nbank,
        "nsext": nsext,
        "npc_out": npc_out,
        "n_bands": n_bands,
        "band_rows": band_rows,
    }
    return in_maps, meta, win_of


def _build_program(meta, N, C, n_cores):
    from concourse import bacc, bass, mybir, tile

    f32 = mybir.dt.float32
    bf16 = mybir.dt.bfloat16
    i32 = mybir.dt.int32
    i16 = mybir.dt.int16
    gtot = meta["gtot"]
    nsub = meta["nsub"]
    nsext = meta["nsext"]
    npc_out = meta["npc_out"]
    band_rows = meta["band_rows"]
    calls = meta["calls"]
    garr = meta["garr"]
    gsext = meta["gsext"]
    started_banks = meta["started_banks"]

    nc = bacc.Bacc("TRN2", target_bir_lowering=False, debug=False,
                   num_devices=n_cores, num_swdge_queues=N_QUEUES,
                   dynamic_dma_scratch_size=32768)
    xs_d = nc.dram_tensor("xs", [N, 2 * C], bf16, kind="ExternalInput")
    gidx_d = nc.dram_tensor("gidx", [P, gtot * 8], i16, kind="ExternalInput")
    dr_d = nc.dram_tensor("dstrel", [P, gtot], f32, kind="ExternalInput")
    xslocT_d = nc.dram_tensor("xslocT", [C, npc_out], f32,
                              kind="ExternalInput")
    dloc_d = nc.dram_tensor("dinvloc", [P, nsub], f32, kind="ExternalInput")
    w_d = nc.dram_tensor("wmat", [C, C], f32, kind="ExternalInput")
    b_d = nc.dram_tensor("bias", [P, C], f32, kind="ExternalInput")
    out_d = nc.dram_tensor("out", [npc_out, C], f32, kind="ExternalOutput")

    # call index -> sextet (for psum tile rotation)
    sext_of_call = []
    for ci, (bb, cg0, cng) in enumerate(calls):
        s = next(s for s in range(nsext)
                 if gsext[s] <= cg0 < gsext[s + 1])
        sext_of_call.append(s)

    with tile.TileContext(nc) as tc:
        with (
            tc.tile_pool(name="const", bufs=1) as cpool,
            tc.tile_pool(name="aux", bufs=1) as apool,
            tc.tile_pool(name="msg", bufs=6) as mpool,
            tc.tile_pool(name="oh", bufs=4) as ohpool,
            tc.tile_pool(name="ohm", bufs=3) as ohmpool,
            tc.tile_pool(name="flush", bufs=3) as fpool,
            tc.tile_pool(name="agg_ps", bufs=2, space="PSUM") as pspool,
            tc.tile_pool(name="out_ps", bufs=2, space="PSUM") as pspool2,
        ):
            iota_i = cpool.tile([P, MW], i32)
            nc.gpsimd.iota(iota_i[:], pattern=[[1, MW]], base=0,
                           channel_multiplier=0)
            iota_f = cpool.tile([P, MW], f32)
            nc.vector.tensor_copy(iota_f[:], iota_i[:])
            wt = cpool.tile([C, C], f32)
            nc.sync.dma_start(out=wt[:], in_=w_d[:])
            bt = cpool.tile([P, C], f32)
            nc.sync.dma_start(out=bt[:], in_=b_d[:])
            gidx_sb = apool.tile([P, gtot * 8], i16)
            dr_sb = apool.tile([P, gtot], f32)
            # chunk the index/dstrel preloads per sextet so the first
            # gather doesn't wait on the whole table
            for s in range(nsext):
                lo8, hi8 = gsext[s] * 8, gsext[s + 1] * 8
                if hi8 > lo8:
                    nc.sync.dma_start(out=gidx_sb[:, lo8:hi8],
                                      in_=gidx_d[:, lo8:hi8])
                    nc.sync.dma_start(out=dr_sb[:, gsext[s]:gsext[s + 1]],
                                      in_=dr_d[:, gsext[s]:gsext[s + 1]])
            dloc_sb = apool.tile([P, nsub], f32)
            nc.sync.dma_start(out=dloc_sb[:], in_=dloc_d[:])
            agg_sb = apool.tile([C, npc_out], f32)
            nc.sync.dma_start(out=agg_sb[:], in_=xslocT_d[:])

            def finals(j):
                # j: subslot == final 128-window
                out_ps = pspool2.tile([P, C], f32)
                nc.tensor.matmul(
                    out_ps[:],
                    lhsT=agg_sb[:, j * SW:(j + 1) * SW],
                    rhs=wt[:],
                    start=True,
                    stop=True,
                )
                out_sb = fpool.tile([P, C], f32)
                nc.vector.scalar_tensor_tensor(
                    out=out_sb[:], in0=out_ps[:],
                    scalar=dloc_sb[:, j:j + 1],
                    in1=bt[:],
                    op0=mybir.AluOpType.mult,
                    op1=mybir.AluOpType.add)
                nc.sync.dma_start(
                    out=out_d[j * SW:(j + 1) * SW, :],
                    in_=out_sb[:])

            def finish_bank(s, bloc, agg):
                # flush this PSUM bank into agg_sb and emit its finals;
                # fires at the bank's last group so the work overlaps the
                # remaining gathers instead of clustering at sextet end
                j0 = s * 2 * SEXT + bloc * BANKSUB
                j1 = min(j0 + BANKSUB, nsub)
                lo = bloc * MW
                hi = lo + (j1 - j0) * SW
                nc.vector.tensor_tensor(
                    out=agg_sb[:, j0 * SW:j1 * SW],
                    in0=agg_sb[:, j0 * SW:j1 * SW],
                    in1=agg[:, lo:hi],
                    op=mybir.AluOpType.add)
                for j in range(j0, j1):
                    finals(j)

            agg = None
            cur_sext = -1
            for ci, (bb, cg0, cng) in enumerate(calls):
                s = sext_of_call[ci]
                if s != cur_sext:
                    agg = pspool.tile([C, 2 * SEXT * SW], f32)
                    cur_sext = s
                msg = mpool.tile([P, KG, 2 * C], bf16)
                lo = bb * band_rows
                hi = min(lo + band_rows, N)
                nc.gpsimd.dma_gather(
                    out_ap=msg[:, :cng, :],
                    in_ap=xs_d[lo:hi, :],
                    idxs_ap=gidx_sb[:, cg0 * 8:(cg0 + cng) * 8],
                    num_idxs=cng * P,
                    num_idxs_reg=cng * P,
                    elem_size=2 * C,
                    single_packet=False,
                    queue_num=ci % N_QUEUES,
                )
                # one-hot builds batched over runs of equal width,
                # interleaved with the consuming matmuls
                oh = None
                batch0 = 0
                nb = 0
                for jj in range(cng):
                    g = cg0 + jj
                    _, kind, off, width, st, sp = garr[g]
                    if jj >= batch0 + nb:
                        batch0 = jj
                        bmax = OH_B if width == SW else OH_BM
                        nb = 1
                        while (nb < bmax and jj + nb < cng
                               and garr[g + nb][3] == width):
                            nb += 1
                        pool = ohpool if width == SW else ohmpool
                        oh = pool.tile([P, bmax, width], bf16)
                        nc.vector.tensor_tensor(
                            out=oh[:, :nb, :],
                            in0=iota_f[:, None, :width].to_broadcast(
                                [P, nb, width]),
                            in1=dr_sb[:, g:g + nb, None].to_broadcast(
                                [P, nb, width]),
                            op=mybir.AluOpType.is_equal,
                        )
                    nc.tensor.matmul(
                        agg[:, off:off + width],
                        lhsT=msg[:, jj, 0:C],
                        rhs=oh[:, jj - batch0, :],
                        start=st,
                        stop=sp,
                    )
                    if sp:
                        finish_bank(s, off // MW, agg)
            # windows whose banks never saw an edge keep the xsloc init
            for s in range(nsext):
                for bloc in range(NBANK_SEXT):
                    if (s, bloc) in started_banks:
                        continue
                    j0 = s * 2 * SEXT + bloc * BANKSUB
                    for j in range(j0, min(j0 + BANKSUB, nsub)):
                        finals(j)
    nc.compile()
    return nc


_PROGRAM_CACHE = {}


def _run(x, edge_index, W, b, n_cores=N_CORES, band_rows=BAND_ROWS,
         trace=False, sim=False, sim_cores=None):
    in_maps, meta, win_of = _prepare(x, edge_index, W, b, n_cores, band_rows)
    key = (tuple(meta["garr"]), tuple(meta["calls"]), x.shape)
    nc = _PROGRAM_CACHE.get(key)
    if nc is None:
        nc = _build_program(meta, x.shape[0], x.shape[1], n_cores)
        _PROGRAM_CACHE[key] = nc

    N, C = x.shape
    nslots = meta["nslots"]

    if sim:
        from concourse.bass_interp import CoreSim
        outs = {}
        for c in (sim_cores if sim_cores is not None else range(n_cores)):
            s = CoreSim(nc)
            for k, v in in_maps[c].items():
                s.tensor(k)[:] = v
            s.simulate()
            outs[c] = np.array(s.tensor("out"))
        exec_ns = None
    else:
        from concourse.bass_utils import run_bass_kernel_spmd
        res = run_bass_kernel_spmd(nc, in_maps, list(range(n_cores)),
                                   trace=trace)
        outs = {c: res.results[c]["out"] for c in range(n_cores)}
        exec_ns = res.exec_time_ns

    out_full = np.zeros((N, C), np.float32)
    for c, co in outs.items():
        for j in range(nslots):
            w = win_of[c, j]
            if w < 0:
                continue
            lo = w * AW
            ws = min(AW, N - lo)
            out_full[lo:lo + ws] = co[j * AW:j * AW + ws]
    return out_full, exec_ns


def kernel(x, edge_index, W, b):
    out, _ = _run(np.asarray(x), np.asarray(edge_index), np.asarray(W),
                  np.asarray(b))
    return out


# revision 25
# speedup vs baseline: 1.1081x; 1.1081x over previous
"""GCNConv Bass kernel for Trainium2, 8-core SPMD.

Math (reference): out = D^-1/2 (A + I) D^-1/2 (x @ W) + b.
Aggregation commutes with the linear layer; with xs = dinv * x pre-scaled:
    out[d] = dinv[d] * ( sum_{e: dst(e)=d} xs[src(e)] + xs[d] ) @ W + b

Sharding: 256-dst windows are bin-packed across the 8 cores (local-search
on per-bank group cost).  SWDGE descriptor emission on GpSimd (~2.9ns per
gathered row, 4 queues) is the wall, so group count is minimized with a
two-tier scheme per (band, psum-bank of 512 dsts): per-128-subslot PURE
groups (one-hot width 128) plus bank-wide MIXED groups (width 512) that
absorb every subslot's leftovers - total groups hit the per-bank lower
bound ceil(max_core bank_edges/128).  Group counts are shared across
cores (SPMD single program); padding uses idx 0 + dst_rel=-1 (the
negative-index skip path is ~2.6x slower, so pads gather row 0 and the
one-hot drops them).

Slots are processed in sextets (6 slots = 12 subslots = 3 PSUM banks)
whose [64f, 1536d] PSUM accumulator stays resident across all 4 source
bands (bands exist because dma_gather indices are int16; 25000 rows of
256B each).  Each 2KB PSUM bank is one accumulation group (bank zeroing
granule): the bank's first group in program order carries start=True.

xs is stored bf16 in a [N, 128] row (left half = features, right half
zero) so each gather descriptor is 256B (hardware minimum) yet messages
arrive in bf16 directly - scatter matmuls run at bf16 PE rates.  One-hot
compares run on f32 iota/dst_rel (exact integers beyond 256) with bf16
output.  Self loops live in agg_sb's DMA-loaded initial value (xsloc^T);
bank flushes add the PSUM cell on top.

Final per 128-window: agg^T @ W matmul (fp32), fused dinv scale + bias
via scalar_tensor_tensor, output DMA.
"""

import numpy as np
import ml_dtypes

BF16 = ml_dtypes.bfloat16

N_CORES = 8
AW = 256  # window width for core assignment / output
SW = 128  # subslot width = pure one-hot width = final window
MW = 512  # mixed one-hot width = PSUM bank width
P = 128
SEXT = 6  # 256-slots per PSUM residency (= 12 subslots = 3 banks)
BANKSUB = 4  # subslots per 2KB PSUM bank (zeroing granule)
NBANK_SEXT = 3
BAND_ROWS = 25000  # int16 gather index limit (256B rows)
KG = 32  # max groups (of 128 edges) per dma_gather call
OH_B = 16  # pure groups per batched one-hot DVE instruction
OH_BM = 4  # mixed groups per batch
N_QUEUES = 4


def _assign_windows(cnt_w, n_cores, bandcnt=None, iters=60000):
    """Bin-pack 256-windows across cores by edge count; refine with a
    local search minimizing sum over (bank = adjacent slot pair, band) of
    ceil(max_core bank_count / 128) (the bank-mixed group lower bound)."""
    nw = len(cnt_w)
    nslots = -(-nw // n_cores)
    order = np.argsort(-cnt_w, kind="stable")
    win_of = np.full((n_cores, nslots), -1, np.int64)
    for j in range(nslots):
        chunk = order[j * n_cores:(j + 1) * n_cores]
        for i, w in enumerate(chunk):
            win_of[(i + j) % n_cores, j] = w

    if bandcnt is None:
        return win_of

    nbands = bandcnt.shape[1]
    bc = np.vstack([bandcnt, np.zeros((1, nbands), bandcnt.dtype)])
    assign = np.where(win_of >= 0, win_of, nw)  # [n_cores, nslots]
    npair = -(-nslots // 2)

    def pair_cost(jp, a=None):
        a = assign if a is None else a
        j0, j1 = 2 * jp, 2 * jp + 1
        tot = bc[a[:, j0]]
        if j1 < nslots:
            tot = tot + bc[a[:, j1]]
        return int(np.ceil(tot.max(axis=0) / P).sum())

    costs = np.array([pair_cost(jp) for jp in range(npair)])
    rng = np.random.default_rng(12345)
    ja = rng.integers(0, nslots, iters)
    jb = rng.integers(0, nslots, iters)
    ca = rng.integers(0, n_cores, iters)
    cb = rng.integers(0, n_cores, iters)
    for t in range(iters):
        j1, j2, c1, c2 = ja[t], jb[t], ca[t], cb[t]
        p1, p2 = j1 // 2, j2 // 2
        if p1 == p2:
            continue
        a = assign.copy()
        a[c1, j1], a[c2, j2] = assign[c2, j2], assign[c1, j1]
        n1, n2 = pair_cost(p1, a), pair_cost(p2, a)
        if n1 + n2 < costs[p1] + costs[p2]:
            assign[c1, j1], assign[c2, j2] = a[c1, j1], a[c2, j2]
            costs[p1], costs[p2] = n1, n2
    return np.where(assign == nw, -1, assign)


def _prepare(x, edge_index, W, b, n_cores, band_rows):
    N, C = x.shape
    n_bands = -(-N // band_rows)
    nw = -(-N // AW)  # global 256-dst windows
    nslots = -(-nw // n_cores)
    nsub = nslots * 2
    nbank = -(-nsub // BANKSUB)
    nsext = -(-nslots // SEXT)
    npc_out = nslots * AW

    row = np.asarray(edge_index[0], dtype=np.int64)
    col = np.asarray(edge_index[1], dtype=np.int64)

    deg = np.bincount(col, minlength=N) + 1  # +1 self loop
    dinv = (1.0 / np.sqrt(deg)).astype(np.float32)
    xs = np.asarray(x, dtype=np.float32) * dinv[:, None]

    w_glob = col // AW
    cnt_w = np.bincount(w_glob, minlength=nw)
    bandcnt = np.bincount(w_glob * n_bands + row // band_rows,
                          minlength=nw * n_bands).reshape(nw, n_bands)
    win_of = _assign_windows(cnt_w, n_cores, bandcnt)
    core_of_w = np.zeros(nw, np.int64)
    slot_of_w = np.zeros(nw, np.int64)
    for c in range(n_cores):
        for j in range(nslots):
            w = win_of[c, j]
            if w >= 0:
                core_of_w[w] = c
                slot_of_w[w] = j

    core = core_of_w[w_glob]
    rel256 = col - w_glob * AW
    sub = slot_of_w[w_glob] * 2 + rel256 // SW
    rel128 = (rel256 % SW).astype(np.float32)
    band = row // band_rows
    sext = sub // (2 * SEXT)

    order = np.lexsort((row, sub, band, sext, core))
    band_s = band[order]
    rel_row_s = (row[order] - band_s * band_rows).astype(np.int16)
    dr_s = rel128[order]
    sub_s = sub[order]

    # edge ranges per (core, band, sub) cell, in sorted order:
    # rank = (core, sext, band, sub) program order
    cell_rank = np.zeros((n_bands, nsub), np.int64)
    r = 0
    for s in range(nsext):
        for bb in range(n_bands):
            for j in range(s * 2 * SEXT, min((s + 1) * 2 * SEXT, nsub)):
                cell_rank[bb, j] = r
                r += 1
    n_cells = n_bands * nsub
    key = core[order] * n_cells + cell_rank[band_s, sub_s]
    cnt_rank = np.bincount(key, minlength=n_cores * n_cells).reshape(
        n_cores, n_cells)
    estart = np.zeros(n_cores * n_cells + 1, np.int64)
    estart[1:] = np.cumsum(cnt_rank.reshape(-1))

    def cell_cnt(c, bb, j):
        return cnt_rank[c, cell_rank[bb, j]]

    # per (band, bank): choose pure group counts per subslot + mixed count
    # minimizing total groups (tie: max pure share)
    from itertools import product
    G_pure = np.zeros((n_bands, nsub), np.int64)
    G_mix = np.zeros((n_bands, nbank), np.int64)
    for bb in range(n_bands):
        for bk in range(nbank):
            subs = list(range(bk * BANKSUB, min((bk + 1) * BANKSUB, nsub)))
            c_mat = np.array([[cell_cnt(c, bb, j) for j in subs]
                              for c in range(n_cores)])
            best = None
            cands = [range(int(c_mat[:, i].min()) // P,
                           int(c_mat[:, i].max()) // P + 1)
                     for i in range(len(subs))]
            for gp in product(*cands):
                spill = np.maximum(c_mat - np.array(gp) * P, 0).sum(axis=1)
                gm = int(np.ceil(spill.max() / P))
                tot = sum(gp) + gm
                sc = (tot, -sum(gp))
                if best is None or sc < best[0]:
                    best = (sc, gp, gm)
            G_pure[bb, subs] = best[1]
            G_mix[bb, bk] = best[2]

    # global group order: (sext, band, bank, [pure sub0..sub3, mixed])
    # garr: (band, kind, psum_off, width, start, stop) per group
    garr = []
    gp_start = np.zeros((n_bands, nsub), np.int64)
    gm_start = np.zeros((n_bands, nbank), np.int64)
    gsext = []  # first group index of each sextet
    pair_first = {}
    pair_last = {}
    for s in range(nsext):
        gsext.append(len(garr))
        for bb in range(n_bands):
            for bk in range(s * NBANK_SEXT,
                            min((s + 1) * NBANK_SEXT, nbank)):
                bloc = bk - s * NBANK_SEXT
                pr = (s, bloc)
                for j in range(bk * BANKSUB,
                               min((bk + 1) * BANKSUB, nsub)):
                    gp_start[bb, j] = len(garr)
                    off = (j - s * 2 * SEXT) * SW
                    for k in range(int(G_pure[bb, j])):
                        pair_first.setdefault(pr, len(garr))
                        pair_last[pr] = len(garr)
                        garr.append([bb, 0, off, SW, False, False])
                gm_start[bb, bk] = len(garr)
                for k in range(int(G_mix[bb, bk])):
                    pair_first.setdefault(pr, len(garr))
                    pair_last[pr] = len(garr)
                    garr.append([bb, 1, bloc * MW, MW, False, False])
    gsext.append(len(garr))
    gtot = len(garr)
    for pr, gi in pair_first.items():
        garr[gi][4] = True
    for pr, gi in pair_last.items():
        garr[gi][5] = True
    started_banks = set(pair_first)

    # calls: chunks of <=KG groups within one (sext, band)
    calls = []
    for s in range(nsext):
        for bb in range(n_bands):
            bk0 = s * NBANK_SEXT
            bk1 = min((s + 1) * NBANK_SEXT, nbank)
            j0 = bk0 * BANKSUB
            g0 = int(gp_start[bb, j0]) if G_pure[bb, j0] > 0 or True else 0
            # first group of (s, bb) = gp_start of first sub (even if its
            # pure count is 0, gp_start still marks the position)
            gend = (int(gm_start[bb, bk1 - 1]) + int(G_mix[bb, bk1 - 1]))
            g = g0
            while g < gend:
                ng = min(KG, gend - g)
                calls.append((bb, g, ng))
                g += ng

    xs_pack = np.zeros((N, 2 * C), dtype=BF16)
    xs_pack[:, :C] = xs.astype(BF16)
    W32 = np.ascontiguousarray(np.asarray(W, dtype=np.float32))
    b32 = np.broadcast_to(np.asarray(b, dtype=np.float32), (P, C)).copy()

    in_maps = []
    for c in range(n_cores):
        # pad idx 0: valid row, gathered but discarded (dst_rel=-1); the
        # negative-index skip path is slower than a real descriptor
        ridx = np.zeros((gtot, P), np.int16)
        drel = np.full((gtot, P), -1.0, np.float32)
        for bb in range(n_bands):
            for bk in range(nbank):
                spill_r = []
                spill_d = []
                for j in range(bk * BANKSUB,
                               min((bk + 1) * BANKSUB, nsub)):
                    k = c * n_cells + cell_rank[bb, j]
                    e0, e1 = estart[k], estart[k + 1]
                    cap = int(G_pure[bb, j]) * P
                    npure = min(int(e1 - e0), cap)
                    g0 = gp_start[bb, j]
                    ridx[g0:g0 + cap // P].reshape(-1)[:npure] = \
                        rel_row_s[e0:e0 + npure]
                    drel[g0:g0 + cap // P].reshape(-1)[:npure] = \
                        dr_s[e0:e0 + npure]
                    if e0 + npure < e1:
                        spill_r.append(rel_row_s[e0 + npure:e1])
                        spill_d.append(dr_s[e0 + npure:e1]
                                       + (j % BANKSUB) * SW)
                gm0, gmn = gm_start[bb, bk], int(G_mix[bb, bk])
                if spill_r:
                    sr = np.concatenate(spill_r)
                    sd = np.concatenate(spill_d)
                    assert len(sr) <= gmn * P, (len(sr), gmn)
                    ridx[gm0:gm0 + gmn].reshape(-1)[:len(sr)] = sr
                    drel[gm0:gm0 + gmn].reshape(-1)[:len(sd)] = sd
        gidx = np.tile(
            ridx.reshape(gtot, 8, 16).transpose(2, 0, 1).reshape(16, gtot * 8),
            (8, 1)).astype(np.int16)

        # agg init = self-loop contribution xs^T for this core's windows
        xslocT = np.zeros((C, npc_out), np.float32)
        dloc = np.zeros(nsub * P, np.float32)
        for j in range(nslots):
            w = win_of[c, j]
            if w < 0:
                continue
            lo = w * AW
            ws = min(AW, N - lo)
            xslocT[:, j * AW:j * AW + ws] = xs[lo:lo + ws].T
            dloc[j * AW:j * AW + ws] = dinv[lo:lo + ws]
        dinvloc = np.ascontiguousarray(dloc.reshape(nsub, P).T)

        in_maps.append({
            "xs": xs_pack,
            "gidx": np.ascontiguousarray(gidx),
            "dstrel": np.ascontiguousarray(drel.T),
            "xslocT": xslocT,
            "dinvloc": dinvloc,
            "wmat": W32,
            "bias": b32,
        })
    meta = {
        "garr": [tuple(g) for g in garr],
        "calls": calls,
        "gsext": gsext,
        "started_banks": started_banks,
        "gtot": gtot,
        "nslots": nslots,
        "nsub": nsub,
        "nbank": nbank,
        "nsext": nsext,
        "npc_out": npc_out,
        "n_bands": n_bands,
        "band_rows": band_rows,
    }
    return in_maps, meta, win_of


def _build_program(meta, N, C, n_cores):
    from concourse import bacc, bass, mybir, tile

    f32 = mybir.dt.float32
    bf16 = mybir.dt.bfloat16
    i32 = mybir.dt.int32
    i16 = mybir.dt.int16
    gtot = meta["gtot"]
    nsub = meta["nsub"]
    nsext = meta["nsext"]
    npc_out = meta["npc_out"]
    band_rows = meta["band_rows"]
    calls = meta["calls"]
    garr = meta["garr"]
    gsext = meta["gsext"]
    started_banks = meta["started_banks"]

    nc = bacc.Bacc("TRN2", target_bir_lowering=False, debug=False,
                   num_devices=n_cores, num_swdge_queues=N_QUEUES,
                   dynamic_dma_scratch_size=32768)
    xs_d = nc.dram_tensor("xs", [N, 2 * C], bf16, kind="ExternalInput")
    gidx_d = nc.dram_tensor("gidx", [P, gtot * 8], i16, kind="ExternalInput")
    dr_d = nc.dram_tensor("dstrel", [P, gtot], f32, kind="ExternalInput")
    xslocT_d = nc.dram_tensor("xslocT", [C, npc_out], f32,
                              kind="ExternalInput")
    dloc_d = nc.dram_tensor("dinvloc", [P, nsub], f32, kind="ExternalInput")
    w_d = nc.dram_tensor("wmat", [C, C], f32, kind="ExternalInput")
    b_d = nc.dram_tensor("bias", [P, C], f32, kind="ExternalInput")
    out_d = nc.dram_tensor("out", [npc_out, C], f32, kind="ExternalOutput")

    # call index -> sextet (for psum tile rotation)
    sext_of_call = []
    for ci, (bb, cg0, cng) in enumerate(calls):
        s = next(s for s in range(nsext)
                 if gsext[s] <= cg0 < gsext[s + 1])
        sext_of_call.append(s)

    with tile.TileContext(nc) as tc:
        with (
            tc.tile_pool(name="const", bufs=1) as cpool,
            tc.tile_pool(name="aux", bufs=1) as apool,
            tc.tile_pool(name="msg", bufs=6) as mpool,
            tc.tile_pool(name="oh", bufs=4) as ohpool,
            tc.tile_pool(name="ohm", bufs=3) as ohmpool,
            tc.tile_pool(name="flush", bufs=3) as fpool,
            tc.tile_pool(name="agg_ps", bufs=2, space="PSUM") as pspool,
            tc.tile_pool(name="out_ps", bufs=2, space="PSUM") as pspool2,
        ):
            iota_i = cpool.tile([P, MW], i32)
            nc.gpsimd.iota(iota_i[:], pattern=[[1, MW]], base=0,
                           channel_multiplier=0)
            iota_f = cpool.tile([P, MW], f32)
            nc.vector.tensor_copy(iota_f[:], iota_i[:])
            wt = cpool.tile([C, C], f32)
            nc.sync.dma_start(out=wt[:], in_=w_d[:])
            bt = cpool.tile([P, C], f32)
            nc.sync.dma_start(out=bt[:], in_=b_d[:])
            gidx_sb = apool.tile([P, gtot * 8], i16)
            dr_sb = apool.tile([P, gtot], f32)
            # chunk the index/dstrel preloads per sextet so the first
            # gather doesn't wait on the whole table
            for s in range(nsext):
                lo8, hi8 = gsext[s] * 8, gsext[s + 1] * 8
                if hi8 > lo8:
                    nc.sync.dma_start(out=gidx_sb[:, lo8:hi8],
                                      in_=gidx_d[:, lo8:hi8])
                    nc.sync.dma_start(out=dr_sb[:, gsext[s]:gsext[s + 1]],
                                      in_=dr_d[:, gsext[s]:gsext[s + 1]])
            dloc_sb = apool.tile([P, nsub], f32)
            nc.sync.dma_start(out=dloc_sb[:], in_=dloc_d[:])
            agg_sb = apool.tile([C, npc_out], f32)
            nc.sync.dma_start(out=agg_sb[:], in_=xslocT_d[:])

            def finals(j):
                # j: subslot == final 128-window
                out_ps = pspool2.tile([P, C], f32)
                nc.tensor.matmul(
                    out_ps[:],
                    lhsT=agg_sb[:, j * SW:(j + 1) * SW],
                    rhs=wt[:],
                    start=True,
                    stop=True,
                )
                out_sb = fpool.tile([P, C], f32)
                nc.vector.scalar_tensor_tensor(
                    out=out_sb[:], in0=out_ps[:],
                    scalar=dloc_sb[:, j:j + 1],
                    in1=bt[:],
                    op0=mybir.AluOpType.mult,
                    op1=mybir.AluOpType.add)
                nc.sync.dma_start(
                    out=out_d[j * SW:(j + 1) * SW, :],
                    in_=out_sb[:])

            def finish_sextet(s, agg):
                j0 = s * 2 * SEXT
                j1 = min(j0 + 2 * SEXT, nsub)
                for bloc in range(NBANK_SEXT):
                    lo = bloc * MW
                    hi = min((bloc + 1) * MW, (j1 - j0) * SW)
                    if hi <= lo:
                        continue
                    if (s, bloc) in started_banks:
                        nc.vector.tensor_tensor(
                            out=agg_sb[:, j0 * SW + lo:j0 * SW + hi],
                            in0=agg_sb[:, j0 * SW + lo:j0 * SW + hi],
                            in1=agg[:, lo:hi],
                            op=mybir.AluOpType.add)
                for j in range(j0, j1):
                    finals(j)

            agg = None
            cur_sext = -1
            for ci, (bb, cg0, cng) in enumerate(calls):
                s = sext_of_call[ci]
                if s != cur_sext:
                    if cur_sext >= 0:
                        finish_sextet(cur_sext, agg)
                    agg = pspool.tile([C, 2 * SEXT * SW], f32)
                    cur_sext = s
                msg = mpool.tile([P, KG, 2 * C], bf16)
                lo = bb * band_rows
                hi = min(lo + band_rows, N)
                nc.gpsimd.dma_gather(
                    out_ap=msg[:, :cng, :],
                    in_ap=xs_d[lo:hi, :],
                    idxs_ap=gidx_sb[:, cg0 * 8:(cg0 + cng) * 8],
                    num_idxs=cng * P,
                    num_idxs_reg=cng * P,
                    elem_size=2 * C,
                    single_packet=False,
                    queue_num=ci % N_QUEUES,
                )
                # one-hot builds batched over runs of equal width,
                # interleaved with the consuming matmuls
                oh = None
                batch0 = 0
                nb = 0
                for jj in range(cng):
                    g = cg0 + jj
                    _, kind, off, width, st, sp = garr[g]
                    if jj >= batch0 + nb:
                        batch0 = jj
                        bmax = OH_B if width == SW else OH_BM
                        nb = 1
                        while (nb < bmax and jj + nb < cng
                               and garr[g + nb][3] == width):
                            nb += 1
                        pool = ohpool if width == SW else ohmpool
                        oh = pool.tile([P, bmax, width], bf16)
                        nc.vector.tensor_tensor(
                            out=oh[:, :nb, :],
                            in0=iota_f[:, None, :width].to_broadcast(
                                [P, nb, width]),
                            in1=dr_sb[:, g:g + nb, None].to_broadcast(
                                [P, nb, width]),
                            op=mybir.AluOpType.is_equal,
                        )
                    nc.tensor.matmul(
                        agg[:, off:off + width],
                        lhsT=msg[:, jj, 0:C],
                        rhs=oh[:, jj - batch0, :],
                        start=st,
                        stop=sp,
                    )
            if cur_sext >= 0:
                finish_sextet(cur_sext, agg)
    nc.compile()
    return nc


_PROGRAM_CACHE = {}


def _run(x, edge_index, W, b, n_cores=N_CORES, band_rows=BAND_ROWS,
         trace=False, sim=False, sim_cores=None):
    in_maps, meta, win_of = _prepare(x, edge_index, W, b, n_cores, band_rows)
    key = (tuple(meta["garr"]), tuple(meta["calls"]), x.shape)
    nc = _PROGRAM_CACHE.get(key)
    if nc is None:
        nc = _build_program(meta, x.shape[0], x.shape[1], n_cores)
        _PROGRAM_CACHE[key] = nc

    N, C = x.shape
    nslots = meta["nslots"]

    if sim:
        from concourse.bass_interp import CoreSim
        outs = {}
        for c in (sim_cores if sim_cores is not None else range(n_cores)):
            s = CoreSim(nc)
            for k, v in in_maps[c].items():
                s.tensor(k)[:] = v
            s.simulate()
            outs[c] = np.array(s.tensor("out"))
        exec_ns = None
    else:
        from concourse.bass_utils import run_bass_kernel_spmd
        res = run_bass_kernel_spmd(nc, in_maps, list(range(n_cores)),
                                   trace=trace)
        outs = {c: res.results[c]["out"] for c in range(n_cores)}
        exec_ns = res.exec_time_ns

    out_full = np.zeros((N, C), np.float32)
    for c, co in outs.items():
        for j in range(nslots):
            w = win_of[c, j]
            if w < 0:
                continue
            lo = w * AW
            ws = min(AW, N - lo)
            out_full[lo:lo + ws] = co[j * AW:j * AW + ws]
    return out_full, exec_ns


def kernel(x, edge_index, W, b):
    out, _ = _run(np.asarray(x), np.asarray(edge_index), np.asarray(W),
                  np.asarray(b))
    return out


# revision 28
# speedup vs baseline: 1.1143x; 1.0056x over previous
"""GCNConv Bass kernel for Trainium2, 8-core SPMD.

Math (reference): out = D^-1/2 (A + I) D^-1/2 (x @ W) + b.
Aggregation commutes with the linear layer; with xs = dinv * x pre-scaled:
    out[d] = dinv[d] * ( sum_{e: dst(e)=d} xs[src(e)] + xs[d] ) @ W + b

Sharding: 256-dst windows are bin-packed across the 8 cores (local-search
on per-bank group cost).  SWDGE descriptor emission on GpSimd (~2.9ns per
gathered row, 4 queues) is the wall, so group count is minimized with a
two-tier scheme per (band, psum-bank of 512 dsts): per-128-subslot PURE
groups (one-hot width 128) plus bank-wide MIXED groups (width 512) that
absorb every subslot's leftovers - total groups hit the per-bank lower
bound ceil(max_core bank_edges/128).  Group counts are shared across
cores (SPMD single program); padding uses idx 0 + dst_rel=-1 (the
negative-index skip path is ~2.6x slower, so pads gather row 0 and the
one-hot drops them).

Slots are processed in sextets (6 slots = 12 subslots = 3 PSUM banks)
whose [64f, 1536d] PSUM accumulator stays resident across all 4 source
bands (bands exist because dma_gather indices are int16; 25000 rows of
256B each).  Each 2KB PSUM bank is one accumulation group (bank zeroing
granule): the bank's first group in program order carries start=True.

xs is stored bf16 in a [N, 128] row (left half = features, right half
zero) so each gather descriptor is 256B (hardware minimum) yet messages
arrive in bf16 directly - scatter matmuls run at bf16 PE rates.  One-hot
compares run on f32 iota/dst_rel (exact integers beyond 256) with bf16
output.  Self loops live in agg_sb's DMA-loaded initial value (xsloc^T);
bank flushes add the PSUM cell on top.

Final per 128-window: agg^T @ W matmul (fp32), fused dinv scale + bias
via scalar_tensor_tensor, output DMA.
"""

import numpy as np
import ml_dtypes

BF16 = ml_dtypes.bfloat16

N_CORES = 8
AW = 256  # window width for core assignment / output
SW = 128  # subslot width = pure one-hot width = final window
MW = 512  # mixed one-hot width = PSUM bank width
P = 128
SEXT = 6  # 256-slots per PSUM residency (= 12 subslots = 3 banks)
BANKSUB = 4  # subslots per 2KB PSUM bank (zeroing granule)
NBANK_SEXT = 3
BAND_ROWS = 25000  # int16 gather index limit (256B rows)
KG = 32  # max groups (of 128 edges) per dma_gather call
OH_B = 16  # pure groups per batched one-hot DVE instruction
OH_BM = 4  # mixed groups per batch
N_QUEUES = 4


def _assign_windows(cnt_w, n_cores, bandcnt=None, iters=60000):
    """Bin-pack 256-windows across cores by edge count; refine with a
    local search minimizing sum over (bank = adjacent slot pair, band) of
    ceil(max_core bank_count / 128) (the bank-mixed group lower bound)."""
    nw = len(cnt_w)
    nslots = -(-nw // n_cores)
    order = np.argsort(-cnt_w, kind="stable")
    win_of = np.full((n_cores, nslots), -1, np.int64)
    for j in range(nslots):
        chunk = order[j * n_cores:(j + 1) * n_cores]
        for i, w in enumerate(chunk):
            win_of[(i + j) % n_cores, j] = w

    if bandcnt is None:
        return win_of

    nbands = bandcnt.shape[1]
    bc = np.vstack([bandcnt, np.zeros((1, nbands), bandcnt.dtype)])
    assign = np.where(win_of >= 0, win_of, nw)  # [n_cores, nslots]
    npair = -(-nslots // 2)

    def pair_cost(jp, a=None):
        a = assign if a is None else a
        j0, j1 = 2 * jp, 2 * jp + 1
        tot = bc[a[:, j0]]
        if j1 < nslots:
            tot = tot + bc[a[:, j1]]
        return int(np.ceil(tot.max(axis=0) / P).sum())

    costs = np.array([pair_cost(jp) for jp in range(npair)])
    rng = np.random.default_rng(12345)
    ja = rng.integers(0, nslots, iters)
    jb = rng.integers(0, nslots, iters)
    ca = rng.integers(0, n_cores, iters)
    cb = rng.integers(0, n_cores, iters)
    for t in range(iters):
        j1, j2, c1, c2 = ja[t], jb[t], ca[t], cb[t]
        p1, p2 = j1 // 2, j2 // 2
        if p1 == p2:
            continue
        a = assign.copy()
        a[c1, j1], a[c2, j2] = assign[c2, j2], assign[c1, j1]
        n1, n2 = pair_cost(p1, a), pair_cost(p2, a)
        if n1 + n2 < costs[p1] + costs[p2]:
            assign[c1, j1], assign[c2, j2] = a[c1, j1], a[c2, j2]
            costs[p1], costs[p2] = n1, n2
    return np.where(assign == nw, -1, assign)


def _prepare(x, edge_index, W, b, n_cores, band_rows):
    N, C = x.shape
    n_bands = -(-N // band_rows)
    nw = -(-N // AW)  # global 256-dst windows
    nslots = -(-nw // n_cores)
    nsub = nslots * 2
    nbank = -(-nsub // BANKSUB)
    nsext = -(-nslots // SEXT)
    npc_out = nslots * AW

    row = np.asarray(edge_index[0], dtype=np.int64)
    col = np.asarray(edge_index[1], dtype=np.int64)

    deg = np.bincount(col, minlength=N) + 1  # +1 self loop
    dinv = (1.0 / np.sqrt(deg)).astype(np.float32)
    xs = np.asarray(x, dtype=np.float32) * dinv[:, None]

    w_glob = col // AW
    cnt_w = np.bincount(w_glob, minlength=nw)
    bandcnt = np.bincount(w_glob * n_bands + row // band_rows,
                          minlength=nw * n_bands).reshape(nw, n_bands)
    win_of = _assign_windows(cnt_w, n_cores, bandcnt)
    core_of_w = np.zeros(nw, np.int64)
    slot_of_w = np.zeros(nw, np.int64)
    for c in range(n_cores):
        for j in range(nslots):
            w = win_of[c, j]
            if w >= 0:
                core_of_w[w] = c
                slot_of_w[w] = j

    core = core_of_w[w_glob]
    rel256 = col - w_glob * AW
    sub = slot_of_w[w_glob] * 2 + rel256 // SW
    rel128 = (rel256 % SW).astype(np.float32)
    band = row // band_rows
    sext = sub // (2 * SEXT)

    order = np.lexsort((row, sub, band, sext, core))
    band_s = band[order]
    rel_row_s = (row[order] - band_s * band_rows).astype(np.int16)
    dr_s = rel128[order]
    sub_s = sub[order]

    # edge ranges per (core, band, sub) cell, in sorted order:
    # rank = (core, sext, band, sub) program order
    cell_rank = np.zeros((n_bands, nsub), np.int64)
    r = 0
    for s in range(nsext):
        for bb in range(n_bands):
            for j in range(s * 2 * SEXT, min((s + 1) * 2 * SEXT, nsub)):
                cell_rank[bb, j] = r
                r += 1
    n_cells = n_bands * nsub
    key = core[order] * n_cells + cell_rank[band_s, sub_s]
    cnt_rank = np.bincount(key, minlength=n_cores * n_cells).reshape(
        n_cores, n_cells)
    estart = np.zeros(n_cores * n_cells + 1, np.int64)
    estart[1:] = np.cumsum(cnt_rank.reshape(-1))

    def cell_cnt(c, bb, j):
        return cnt_rank[c, cell_rank[bb, j]]

    # per (band, bank): choose pure group counts per subslot + mixed count
    # minimizing total groups (tie: max pure share)
    from itertools import product
    G_pure = np.zeros((n_bands, nsub), np.int64)
    G_mix = np.zeros((n_bands, nbank), np.int64)
    for bb in range(n_bands):
        for bk in range(nbank):
            subs = list(range(bk * BANKSUB, min((bk + 1) * BANKSUB, nsub)))
            c_mat = np.array([[cell_cnt(c, bb, j) for j in subs]
                              for c in range(n_cores)])
            best = None
            cands = [range(int(c_mat[:, i].min()) // P,
                           int(c_mat[:, i].max()) // P + 1)
                     for i in range(len(subs))]
            for gp in product(*cands):
                spill = np.maximum(c_mat - np.array(gp) * P, 0).sum(axis=1)
                gm = int(np.ceil(spill.max() / P))
                tot = sum(gp) + gm
                sc = (tot, -sum(gp))
                if best is None or sc < best[0]:
                    best = (sc, gp, gm)
            G_pure[bb, subs] = best[1]
            G_mix[bb, bk] = best[2]

    # global group order: (sext, band, bank, [pure sub0..sub3, mixed])
    # garr: (band, kind, psum_off, width, start, stop) per group
    garr = []
    gp_start = np.zeros((n_bands, nsub), np.int64)
    gm_start = np.zeros((n_bands, nbank), np.int64)
    gsext = []  # first group index of each sextet
    pair_first = {}
    pair_last = {}
    for s in range(nsext):
        gsext.append(len(garr))
        for bb in range(n_bands):
            for bk in range(s * NBANK_SEXT,
                            min((s + 1) * NBANK_SEXT, nbank)):
                bloc = bk - s * NBANK_SEXT
                pr = (s, bloc)
                for j in range(bk * BANKSUB,
                               min((bk + 1) * BANKSUB, nsub)):
                    gp_start[bb, j] = len(garr)
                    off = (j - s * 2 * SEXT) * SW
                    for k in range(int(G_pure[bb, j])):
                        pair_first.setdefault(pr, len(garr))
                        pair_last[pr] = len(garr)
                        garr.append([bb, 0, off, SW, False, False])
                gm_start[bb, bk] = len(garr)
                for k in range(int(G_mix[bb, bk])):
                    pair_first.setdefault(pr, len(garr))
                    pair_last[pr] = len(garr)
                    garr.append([bb, 1, bloc * MW, MW, False, False])
    gsext.append(len(garr))
    gtot = len(garr)
    for pr, gi in pair_first.items():
        garr[gi][4] = True
    for pr, gi in pair_last.items():
        garr[gi][5] = True
    started_banks = set(pair_first)

    # calls: chunks of <=KG groups within one (sext, band)
    calls = []
    for s in range(nsext):
        for bb in range(n_bands):
            bk0 = s * NBANK_SEXT
            bk1 = min((s + 1) * NBANK_SEXT, nbank)
            j0 = bk0 * BANKSUB
            g0 = int(gp_start[bb, j0]) if G_pure[bb, j0] > 0 or True else 0
            # first group of (s, bb) = gp_start of first sub (even if its
            # pure count is 0, gp_start still marks the position)
            gend = (int(gm_start[bb, bk1 - 1]) + int(G_mix[bb, bk1 - 1]))
            g = g0
            while g < gend:
                ng = min(KG, gend - g)
                calls.append((bb, g, ng))
                g += ng

    xs_pack = np.zeros((N, 2 * C), dtype=BF16)
    xs_pack[:, :C] = xs.astype(BF16)
    W32 = np.ascontiguousarray(np.asarray(W, dtype=np.float32))
    b32 = np.broadcast_to(np.asarray(b, dtype=np.float32), (P, C)).copy()

    in_maps = []
    for c in range(n_cores):
        # pad idx 0: valid row, gathered but discarded (dst_rel=-1); the
        # negative-index skip path is slower than a real descriptor
        ridx = np.zeros((gtot, P), np.int16)
        drel = np.full((gtot, P), -1.0, np.float32)
        for bb in range(n_bands):
            for bk in range(nbank):
                spill_r = []
                spill_d = []
                for j in range(bk * BANKSUB,
                               min((bk + 1) * BANKSUB, nsub)):
                    k = c * n_cells + cell_rank[bb, j]
                    e0, e1 = estart[k], estart[k + 1]
                    cap = int(G_pure[bb, j]) * P
                    npure = min(int(e1 - e0), cap)
                    g0 = gp_start[bb, j]
                    ridx[g0:g0 + cap // P].reshape(-1)[:npure] = \
                        rel_row_s[e0:e0 + npure]
                    drel[g0:g0 + cap // P].reshape(-1)[:npure] = \
                        dr_s[e0:e0 + npure]
                    if e0 + npure < e1:
                        spill_r.append(rel_row_s[e0 + npure:e1])
                        spill_d.append(dr_s[e0 + npure:e1]
                                       + (j % BANKSUB) * SW)
                gm0, gmn = gm_start[bb, bk], int(G_mix[bb, bk])
                if spill_r:
                    sr = np.concatenate(spill_r)
                    sd = np.concatenate(spill_d)
                    assert len(sr) <= gmn * P, (len(sr), gmn)
                    ridx[gm0:gm0 + gmn].reshape(-1)[:len(sr)] = sr
                    drel[gm0:gm0 + gmn].reshape(-1)[:len(sd)] = sd
        gidx = np.tile(
            ridx.reshape(gtot, 8, 16).transpose(2, 0, 1).reshape(16, gtot * 8),
            (8, 1)).astype(np.int16)

        # agg init = self-loop contribution xs^T for this core's windows
        xslocT = np.zeros((C, npc_out), np.float32)
        dloc = np.zeros(nsub * P, np.float32)
        for j in range(nslots):
            w = win_of[c, j]
            if w < 0:
                continue
            lo = w * AW
            ws = min(AW, N - lo)
            xslocT[:, j * AW:j * AW + ws] = xs[lo:lo + ws].T
            dloc[j * AW:j * AW + ws] = dinv[lo:lo + ws]
        dinvloc = np.ascontiguousarray(dloc.reshape(nsub, P).T)

        in_maps.append({
            "xs": xs_pack,
            "gidx": np.ascontiguousarray(gidx),
            "dstrel": np.ascontiguousarray(drel.T),
            "xslocT": xslocT,
            "dinvloc": dinvloc,
            "wmat": W32,
            "bias": b32,
        })
    meta = {
        "garr": [tuple(g) for g in garr],
        "calls": calls,
        "gsext": gsext,
        "started_banks": started_banks,
        "gtot": gtot,
        "nslots": nslots,
        "nsub": nsub,
        "nbank": nbank,
        "nsext": nsext,
        "npc_out": npc_out,
        "n_bands": n_bands,
        "band_rows": band_rows,
    }
    return in_maps, meta, win_of


def _build_program(meta, N, C, n_cores):
    from concourse import bacc, bass, mybir, tile

    f32 = mybir.dt.float32
    bf16 = mybir.dt.bfloat16
    i32 = mybir.dt.int32
    i16 = mybir.dt.int16
    gtot = meta["gtot"]
    nsub = meta["nsub"]
    nsext = meta["nsext"]
    npc_out = meta["npc_out"]
    band_rows = meta["band_rows"]
    calls = meta["calls"]
    garr = meta["garr"]
    gsext = meta["gsext"]
    started_banks = meta["started_banks"]

    nc = bacc.Bacc("TRN2", target_bir_lowering=False, debug=False,
                   num_devices=n_cores, num_swdge_queues=N_QUEUES,
                   dynamic_dma_scratch_size=32768)
    xs_d = nc.dram_tensor("xs", [N, 2 * C], bf16, kind="ExternalInput")
    gidx_d = nc.dram_tensor("gidx", [P, gtot * 8], i16, kind="ExternalInput")
    dr_d = nc.dram_tensor("dstrel", [P, gtot], f32, kind="ExternalInput")
    xslocT_d = nc.dram_tensor("xslocT", [C, npc_out], f32,
                              kind="ExternalInput")
    dloc_d = nc.dram_tensor("dinvloc", [P, nsub], f32, kind="ExternalInput")
    w_d = nc.dram_tensor("wmat", [C, C], f32, kind="ExternalInput")
    b_d = nc.dram_tensor("bias", [P, C], f32, kind="ExternalInput")
    out_d = nc.dram_tensor("out", [npc_out, C], f32, kind="ExternalOutput")

    # call index -> sextet (for psum tile rotation)
    sext_of_call = []
    for ci, (bb, cg0, cng) in enumerate(calls):
        s = next(s for s in range(nsext)
                 if gsext[s] <= cg0 < gsext[s + 1])
        sext_of_call.append(s)

    with tile.TileContext(nc) as tc:
        with (
            tc.tile_pool(name="const", bufs=1) as cpool,
            tc.tile_pool(name="aux", bufs=1) as apool,
            tc.tile_pool(name="msg", bufs=6) as mpool,
            tc.tile_pool(name="oh", bufs=6) as ohpool,
            tc.tile_pool(name="ohm", bufs=3) as ohmpool,
            tc.tile_pool(name="flush", bufs=3) as fpool,
            tc.tile_pool(name="agg_ps", bufs=2, space="PSUM") as pspool,
            tc.tile_pool(name="out_ps", bufs=2, space="PSUM") as pspool2,
        ):
            iota_i = cpool.tile([P, MW], i32)
            nc.gpsimd.iota(iota_i[:], pattern=[[1, MW]], base=0,
                           channel_multiplier=0)
            iota_f = cpool.tile([P, MW], f32)
            nc.vector.tensor_copy(iota_f[:], iota_i[:])
            wt = cpool.tile([C, C], f32)
            nc.sync.dma_start(out=wt[:], in_=w_d[:])
            bt = cpool.tile([P, C], f32)
            nc.sync.dma_start(out=bt[:], in_=b_d[:])
            gidx_sb = apool.tile([P, gtot * 8], i16)
            dr_sb = apool.tile([P, gtot], f32)
            # chunk the index/dstrel preloads per sextet so the first
            # gather doesn't wait on the whole table
            for s in range(nsext):
                lo8, hi8 = gsext[s] * 8, gsext[s + 1] * 8
                if hi8 > lo8:
                    nc.sync.dma_start(out=gidx_sb[:, lo8:hi8],
                                      in_=gidx_d[:, lo8:hi8])
                    nc.sync.dma_start(out=dr_sb[:, gsext[s]:gsext[s + 1]],
                                      in_=dr_d[:, gsext[s]:gsext[s + 1]])
            dloc_sb = apool.tile([P, nsub], f32)
            nc.sync.dma_start(out=dloc_sb[:], in_=dloc_d[:])
            agg_sb = apool.tile([C, npc_out], f32)
            nc.sync.dma_start(out=agg_sb[:], in_=xslocT_d[:])

            def finals(j):
                # j: subslot == final 128-window
                out_ps = pspool2.tile([P, C], f32)
                nc.tensor.matmul(
                    out_ps[:],
                    lhsT=agg_sb[:, j * SW:(j + 1) * SW],
                    rhs=wt[:],
                    start=True,
                    stop=True,
                )
                out_sb = fpool.tile([P, C], f32)
                nc.vector.scalar_tensor_tensor(
                    out=out_sb[:], in0=out_ps[:],
                    scalar=dloc_sb[:, j:j + 1],
                    in1=bt[:],
                    op0=mybir.AluOpType.mult,
                    op1=mybir.AluOpType.add)
                nc.sync.dma_start(
                    out=out_d[j * SW:(j + 1) * SW, :],
                    in_=out_sb[:])

            def finish_sextet(s, agg):
                j0 = s * 2 * SEXT
                j1 = min(j0 + 2 * SEXT, nsub)
                for bloc in range(NBANK_SEXT):
                    lo = bloc * MW
                    hi = min((bloc + 1) * MW, (j1 - j0) * SW)
                    if hi <= lo:
                        continue
                    if (s, bloc) in started_banks:
                        nc.vector.tensor_tensor(
                            out=agg_sb[:, j0 * SW + lo:j0 * SW + hi],
                            in0=agg_sb[:, j0 * SW + lo:j0 * SW + hi],
                            in1=agg[:, lo:hi],
                            op=mybir.AluOpType.add)
                for j in range(j0, j1):
                    finals(j)

            agg = None
            cur_sext = -1
            for ci, (bb, cg0, cng) in enumerate(calls):
                s = sext_of_call[ci]
                if s != cur_sext:
                    if cur_sext >= 0:
                        finish_sextet(cur_sext, agg)
                    agg = pspool.tile([C, 2 * SEXT * SW], f32)
                    cur_sext = s
                msg = mpool.tile([P, KG, 2 * C], bf16)
                lo = bb * band_rows
                hi = min(lo + band_rows, N)
                nc.gpsimd.dma_gather(
                    out_ap=msg[:, :cng, :],
                    in_ap=xs_d[lo:hi, :],
                    idxs_ap=gidx_sb[:, cg0 * 8:(cg0 + cng) * 8],
                    num_idxs=cng * P,
                    num_idxs_reg=cng * P,
                    elem_size=2 * C,
                    single_packet=False,
                    queue_num=ci % N_QUEUES,
                )
                # one-hot builds batched over runs of equal width,
                # interleaved with the consuming matmuls
                oh = None
                batch0 = 0
                nb = 0
                for jj in range(cng):
                    g = cg0 + jj
                    _, kind, off, width, st, sp = garr[g]
                    if jj >= batch0 + nb:
                        batch0 = jj
                        bmax = OH_B if width == SW else OH_BM
                        nb = 1
                        while (nb < bmax and jj + nb < cng
                               and garr[g + nb][3] == width):
                            nb += 1
                        pool = ohpool if width == SW else ohmpool
                        oh = pool.tile([P, bmax, width], bf16)
                        nc.vector.tensor_tensor(
                            out=oh[:, :nb, :],
                            in0=iota_f[:, None, :width].to_broadcast(
                                [P, nb, width]),
                            in1=dr_sb[:, g:g + nb, None].to_broadcast(
                                [P, nb, width]),
                            op=mybir.AluOpType.is_equal,
                        )
                    nc.tensor.matmul(
                        agg[:, off:off + width],
                        lhsT=msg[:, jj, 0:C],
                        rhs=oh[:, jj - batch0, :],
                        start=st,
                        stop=sp,
                    )
            if cur_sext >= 0:
                finish_sextet(cur_sext, agg)
    nc.compile()
    return nc


_PROGRAM_CACHE = {}


def _run(x, edge_index, W, b, n_cores=N_CORES, band_rows=BAND_ROWS,
         trace=False, sim=False, sim_cores=None):
    in_maps, meta, win_of = _prepare(x, edge_index, W, b, n_cores, band_rows)
    key = (tuple(meta["garr"]), tuple(meta["calls"]), x.shape)
    nc = _PROGRAM_CACHE.get(key)
    if nc is None:
        nc = _build_program(meta, x.shape[0], x.shape[1], n_cores)
        _PROGRAM_CACHE[key] = nc

    N, C = x.shape
    nslots = meta["nslots"]

    if sim:
        from concourse.bass_interp import CoreSim
        outs = {}
        for c in (sim_cores if sim_cores is not None else range(n_cores)):
            s = CoreSim(nc)
            for k, v in in_maps[c].items():
                s.tensor(k)[:] = v
            s.simulate()
            outs[c] = np.array(s.tensor("out"))
        exec_ns = None
    else:
        from concourse.bass_utils import run_bass_kernel_spmd
        res = run_bass_kernel_spmd(nc, in_maps, list(range(n_cores)),
                                   trace=trace)
        outs = {c: res.results[c]["out"] for c in range(n_cores)}
        exec_ns = res.exec_time_ns

    out_full = np.zeros((N, C), np.float32)
    for c, co in outs.items():
        for j in range(nslots):
            w = win_of[c, j]
            if w < 0:
                continue
            lo = w * AW
            ws = min(AW, N - lo)
            out_full[lo:lo + ws] = co[j * AW:j * AW + ws]
    return out_full, exec_ns


def kernel(x, edge_index, W, b):
    out, _ = _run(np.asarray(x), np.asarray(edge_index), np.asarray(W),
                  np.asarray(b))
    return out
